# revision 10
# baseline (speedup 1.0000x reference)
"""Bahdanau additive attention on 8 Trainium2 NeuronCores.

Problem (per reference):
    pq     = query @ Wq.T + bq                         [B, A]
    pk     = einsum("bsk,ak->bsa", keys, Wk) + bk      [B, S, A]
    scores = einsum("bsa,a->bs", tanh(pq[:,None,:] + pk), Ws)
    attn   = softmax(scores, axis=1)                   [B, S]
    context= einsum("bs,bsv->bv", attn, values)        [B, V]
    returns (context, attn)

B=64, S=1024, QD=KD=VD=AD=1024, fp32.

Sharding: data-parallel over batch across 8 cores (8 batches/core),
weights replicated. No collectives.

Per-core design:
  - All matmuls in float32r (single-pass fp32 at full PE rate; inputs
    must be produced "rounded to f32r" — casts ride existing copies,
    activations, and gpsimd cast-DMAs).
  - keys arrive [s, k]; PE-transposed (fp32, exact) to keysT [k, s].
    Four [128,128] transposes share one PSUM bank; one [128,512] copy
    (alternating DVE/ACT) moves+casts each to SBUF.
  - pk.T psum tiles [a=128, s=512] = WkT @ keysT; the two s-halves are
    computed back-to-back with the same stationary WkT tile to amortize
    the f32r self-weight-load.
  - tanh fused on ScalarE (bias = (pq+bq+bk)[a] per-partition), output
    written directly as f32r.
  - scores [1, 512] = Ws.T @ tanh with Ws as the 1-column stationary
    (trivial weight load) and tanh as the moving operand.
  - softmax in natural layout on one partition: exp via ScalarE with
    accum_out giving the denominator for free; no max-subtraction
    (|scores| <= ||Ws||_1 <= 32 so fp32 exp cannot overflow).
  - attn.T [s=128, 8] for the context matmul is produced by a 4KB
    DRAM-bounce scatter DMA (gpsimd, casting to f32r); the context
    matmuls of batch b are emitted in the middle of batch b+1's PE
    stream so the bounce latency is hidden.
  - context [1, v=512] = attnT @ values, values in natural [s, v]
    layout cast to f32r during their gpsimd DMA load.
"""

import sys

if "/opt/trn_rl_repo" not in sys.path:
    sys.path.insert(0, "/opt/trn_rl_repo")

import numpy as np
from contextlib import ExitStack

import concourse.bass as bass
import concourse.tile as tile
from concourse import bacc, mybir
from concourse.bass_utils import run_bass_kernel_spmd
from concourse.masks import make_identity

F32 = mybir.dt.float32
F32R = mybir.dt.float32r
AF = mybir.ActivationFunctionType

NCORES = 8
B, S, D = 64, 1024, 1024  # D = QD = KD = VD = AD
NB = B // NCORES          # local batches per core
P = 128
KC = D // P               # 8 contraction chunks
AT = D // P               # 8 a-tiles
SB = S // P               # 8 s-blocks of 128
SH = S // 512             # 2 s-halves of 512


def _build_nc(repeat=1):
    nc = bacc.Bacc("TRN2", target_bir_lowering=False, debug=False)

    q_d = nc.dram_tensor("query_l", [NB, D], F32, kind="ExternalInput").ap()
    k_d = nc.dram_tensor("keys_l", [NB, S, D], F32, kind="ExternalInput").ap()
    v_d = nc.dram_tensor("values_l", [NB, S, D], F32, kind="ExternalInput").ap()
    wq_d = nc.dram_tensor("Wq", [D, D], F32, kind="ExternalInput").ap()
    wk_d = nc.dram_tensor("Wk", [D, D], F32, kind="ExternalInput").ap()
    bq_d = nc.dram_tensor("bq", [D], F32, kind="ExternalInput").ap()
    bk_d = nc.dram_tensor("bk", [D], F32, kind="ExternalInput").ap()
    ws_d = nc.dram_tensor("Ws", [D], F32, kind="ExternalInput").ap()
    ctx_d = nc.dram_tensor("context_l", [NB, D], F32, kind="ExternalOutput").ap()
    attn_d = nc.dram_tensor("attn_l", [NB, S], F32, kind="ExternalOutput").ap()

    with tile.TileContext(nc) as tc, ExitStack() as ctx:
        const = ctx.enter_context(tc.tile_pool(name="const", bufs=1))
        tpsum = ctx.enter_context(
            tc.tile_pool(name="tpsum", bufs=2, space=bass.MemorySpace.PSUM)
        )
        mpsum = ctx.enter_context(
            tc.tile_pool(name="mpsum", bufs=4, space=bass.MemorySpace.PSUM)
        )
        spsum = ctx.enter_context(
            tc.tile_pool(name="spsum", bufs=2, space=bass.MemorySpace.PSUM)
        )

        ident = const.tile([P, P], F32, tag="ident")
        make_identity(nc, ident)

        # Ws -> [p, at] fp32 -> f32r
        ws_f = const.tile([P, AT], F32, tag="ws_f")
        nc.sync.dma_start(ws_f, ws_d.rearrange("(a p) -> p a", p=P))
        ws_r = const.tile([P, AT], F32R, tag="ws_r")
        nc.vector.tensor_copy(ws_r, ws_f)

        # bq + bk -> [p, at] fp32
        bq_f = const.tile([P, AT], F32, tag="bq_f")
        bk_f = const.tile([P, AT], F32, tag="bk_f")
        nc.sync.dma_start(bq_f, bq_d.rearrange("(a p) -> p a", p=P))
        nc.sync.dma_start(bk_f, bk_d.rearrange("(a p) -> p a", p=P))
        bqk_f = const.tile([P, AT], F32, tag="bqk_f")
        nc.vector.tensor_add(bqk_f, bq_f, bk_f)

        # WkT (resident): WkT_all[:, kc, a] = Wk[a, kc*128 + p]
        WkT_all = const.tile([P, KC, D], F32R, tag="WkT_all")
        bias_all = const.tile([P, AT, NB], F32, tag="bias_all")  # pq+bq+bk [p, at, b]

        with ExitStack() as setup_ctx:
            wstage = setup_ctx.enter_context(tc.tile_pool(name="wstage", bufs=2))
            wqpool = setup_ctx.enter_context(tc.tile_pool(name="wqpool", bufs=1))

            for at in range(AT):
                wk_nat = wstage.tile([P, D], F32, tag="wnat")
                nc.sync.dma_start(wk_nat, wk_d[at * P : (at + 1) * P, :])
                for kc2 in range(KC // 2):
                    pst = tpsum.tile([P, 2, P], F32, tag="tp")
                    for h in range(2):
                        kc = 2 * kc2 + h
                        nc.tensor.transpose(
                            pst[:, h, :], wk_nat[:, kc * P : (kc + 1) * P], ident
                        )
                    for h in range(2):
                        kc = 2 * kc2 + h
                        dst = WkT_all[:, kc, at * P : (at + 1) * P]
                        if kc2 % 2 == 0:
                            nc.vector.tensor_copy(dst, pst[:, h, :])
                        else:
                            nc.scalar.copy(dst, pst[:, h, :])

            WqT_all = wqpool.tile([P, KC, D], F32R, tag="WqT_all")
            for at in range(AT):
                wq_nat = wstage.tile([P, D], F32, tag="wnat")
                nc.sync.dma_start(wq_nat, wq_d[at * P : (at + 1) * P, :])
                for kc2 in range(KC // 2):
                    pst = tpsum.tile([P, 2, P], F32, tag="tp")
                    for h in range(2):
                        qc = 2 * kc2 + h
                        nc.tensor.transpose(
                            pst[:, h, :], wq_nat[:, qc * P : (qc + 1) * P], ident
                        )
                    for h in range(2):
                        qc = 2 * kc2 + h
                        dst = WqT_all[:, qc, at * P : (at + 1) * P]
                        if kc2 % 2 == 0:
                            nc.vector.tensor_copy(dst, pst[:, h, :])
                        else:
                            nc.scalar.copy(dst, pst[:, h, :])

            # queryT [p(q), qc, b] f32r via PE transposes of query [NB, D]
            q_nat = wqpool.tile([NB, D], F32, tag="q_nat")
            nc.sync.dma_start(q_nat, q_d)
            qT = wqpool.tile([P, KC, NB], F32R, tag="qT")
            for qc in range(KC):
                pst = tpsum.tile([P, NB], F32, tag="tp")
                nc.tensor.transpose(
                    pst, q_nat[0:NB, qc * P : (qc + 1) * P], ident[0:NB, 0:NB]
                )
                nc.vector.tensor_copy(qT[:, qc, :], pst)

            # pqT [a, b] per a-tile; bias_all = pqT + (bq + bk)
            for at in range(AT):
                pqp = tpsum.tile([P, NB], F32, tag="tp")
                for qc in range(KC):
                    nc.tensor.matmul(
                        pqp,
                        WqT_all[:, qc, at * P : (at + 1) * P],
                        qT[:, qc, :],
                        start=(qc == 0),
                        stop=(qc == KC - 1),
                    )
                nc.vector.tensor_scalar_add(
                    bias_all[:, at, :], pqp, bqk_f[:, at : at + 1]
                )

        # ---- main loop over local batches ----
        kpool = ctx.enter_context(tc.tile_pool(name="kpool", bufs=4))
        ktpool = ctx.enter_context(tc.tile_pool(name="ktpool", bufs=2))
        thpool = ctx.enter_context(tc.tile_pool(name="thpool", bufs=16))
        vpool = ctx.enter_context(tc.tile_pool(name="vpool", bufs=10))
        smpool = ctx.enter_context(tc.tile_pool(name="smpool", bufs=2))
        outpool = ctx.enter_context(tc.tile_pool(name="outpool", bufs=2))

        rep_ctx = ExitStack()
        if repeat > 1:
            rep_ctx.enter_context(tc.For_i(0, repeat, 1))

        pending_ctx = None  # (attnT_r, vals, b) of previous batch

        def emit_ctx(pend):
            attnT_r, vals_, b_ = pend
            for vc in range(2):
                cps = spsum.tile([1, 512], F32, tag="sc")
                for sb in range(SB):
                    nc.tensor.matmul(
                        cps,
                        attnT_r[:, sb : sb + 1],
                        vals_[sb][:, vc * 512 : (vc + 1) * 512],
                        start=(sb == 0),
                        stop=(sb == SB - 1),
                    )
                ctx_sb = outpool.tile([1, 512], F32, tag="ctx_sb")
                nc.vector.tensor_copy(ctx_sb, cps)
                nc.sync.dma_start(ctx_d[b_, vc * 512 : (vc + 1) * 512], ctx_sb)

        for b in range(NB):
            # values prefetch (cast to f32r on the way in via SWDGE)
            vals = []
            for sb in range(SB):
                vt = vpool.tile([P, D], F32R, tag="vals")
                nc.gpsimd.dma_start(vt, v_d[b, sb * P : (sb + 1) * P, :])
                vals.append(vt)

            # keys load + transpose into keysT [k, s] per s-half
            kts = []
            for sh in range(SH):
                kt = ktpool.tile([P, KC, 512], F32R, tag="kt")
                knats = []
                for sb4 in range(4):
                    s0 = sh * 512 + sb4 * P
                    knat = kpool.tile([P, D], F32, tag="knat")
                    nc.sync.dma_start(knat, k_d[b, s0 : s0 + P, :])
                    knats.append(knat)
                for kc in range(KC):
                    pst = tpsum.tile([P, 512], F32, tag="tp")
                    for sb4 in range(4):
                        nc.tensor.transpose(
                            pst[:, sb4 * P : (sb4 + 1) * P],
                            knats[sb4][:, kc * P : (kc + 1) * P],
                            ident,
                        )
                    if kc % 2 == 0:
                        nc.vector.tensor_copy(kt[:, kc, :], pst)
                    else:
                        nc.scalar.copy(kt[:, kc, :], pst)
                kts.append(kt)

            # pk.T + tanh; s-halves paired to amortize weight loads
            th = [[None] * AT for _ in range(SH)]
            for at in range(AT):
                if at == 2 and pending_ctx is not None:
                    emit_ctx(pending_ctx)
                    pending_ctx = None
                mps = [
                    mpsum.tile([P, 512], F32, tag="mp", name=f"mp_{b}_{at}_{sh}")
                    for sh in range(SH)
                ]
                for kc in range(KC):
                    w = WkT_all[:, kc, at * P : (at + 1) * P]
                    for sh in range(SH):
                        nc.tensor.matmul(
                            mps[sh],
                            w,
                            kts[sh][:, kc, :],
                            start=(kc == 0),
                            stop=(kc == KC - 1),
                        )
                for sh in range(SH):
                    t = thpool.tile([P, 512], F32R, tag="th")
                    nc.scalar.activation(
                        t, mps[sh], AF.Tanh, bias=bias_all[:, at, b : b + 1]
                    )
                    th[sh][at] = t

            # scores [1, 512] per s-half: Ws (1-col stationary) vs tanh moving
            scores_sb = smpool.tile([1, S], F32, tag="scores")
            for sh in range(SH):
                scp = spsum.tile([1, 512], F32, tag="sc")
                for at in range(AT):
                    nc.tensor.matmul(
                        scp,
                        ws_r[:, at : at + 1],
                        th[sh][at],
                        start=(at == 0),
                        stop=(at == AT - 1),
                    )
                nc.vector.tensor_copy(scores_sb[:, sh * 512 : (sh + 1) * 512], scp)

            # softmax on one partition; denominator via accum_out
            exp_sb = smpool.tile([1, S], F32, tag="exp")
            den = smpool.tile([1, 1], F32, tag="den")
            nc.scalar.activation(exp_sb, scores_sb, AF.Exp, accum_out=den)
            rden = smpool.tile([1, 1], F32, tag="rden")
            nc.vector.reciprocal(rden, den)
            attn_f = outpool.tile([1, S], F32, tag="attn_f")
            nc.vector.tensor_scalar_mul(attn_f, exp_sb, rden)
            nc.sync.dma_start(attn_d[b, :], attn_f)

            # attn.T via DRAM bounce (scatter + f32r cast on SWDGE)
            attnT_r = smpool.tile([P, SB], F32R, tag="attnT")
            nc.gpsimd.dma_start(
                attnT_r, attn_d[b, :].rearrange("(sb p) -> p sb", p=P)
            )

            pending_ctx = (attnT_r, vals, b)

        emit_ctx(pending_ctx)
        rep_ctx.close()

    nc.compile()
    if not nc.is_finalized():
        nc.finalize()
    return nc


_NC_CACHE = None


def _get_nc():
    global _NC_CACHE
    if _NC_CACHE is None:
        _NC_CACHE = _build_nc()
    return _NC_CACHE


def kernel(query, keys, values, Wq, bq, Wk, bk, Ws, **kw):
    query = np.ascontiguousarray(np.asarray(query, dtype=np.float32))
    keys = np.ascontiguousarray(np.asarray(keys, dtype=np.float32))
    values = np.ascontiguousarray(np.asarray(values, dtype=np.float32))
    Wq = np.ascontiguousarray(np.asarray(Wq, dtype=np.float32))
    Wk = np.ascontiguousarray(np.asarray(Wk, dtype=np.float32))
    bq = np.ascontiguousarray(np.asarray(bq, dtype=np.float32))
    bk = np.ascontiguousarray(np.asarray(bk, dtype=np.float32))
    Ws = np.ascontiguousarray(np.asarray(Ws, dtype=np.float32))

    nc = _get_nc()
    in_maps = []
    for c in range(NCORES):
        lo, hi = c * NB, (c + 1) * NB
        in_maps.append(
            {
                "query_l": query[lo:hi],
                "keys_l": keys[lo:hi],
                "values_l": values[lo:hi],
                "Wq": Wq,
                "Wk": Wk,
                "bq": bq,
                "bk": bk,
                "Ws": Ws,
            }
        )
    res = run_bass_kernel_spmd(nc, in_maps, core_ids=list(range(NCORES)))
    context = np.concatenate([r["context_l"] for r in res.results], axis=0)
    attn = np.concatenate([r["attn_l"] for r in res.results], axis=0)
    return context, attn


# revision 11
# speedup vs baseline: 1.0895x; 1.0895x over previous
"""Bahdanau additive attention on 8 Trainium2 NeuronCores.

Problem (per reference):
    pq     = query @ Wq.T + bq                         [B, A]
    pk     = einsum("bsk,ak->bsa", keys, Wk) + bk      [B, S, A]
    scores = einsum("bsa,a->bs", tanh(pq[:,None,:] + pk), Ws)
    attn   = softmax(scores, axis=1)                   [B, S]
    context= einsum("bs,bsv->bv", attn, values)        [B, V]
    returns (context, attn)

B=64, S=1024, QD=KD=VD=AD=1024, fp32.

Sharding: data-parallel over batch across 8 cores (8 batches/core),
weights replicated. No collectives.

Per-core design:
  - All matmuls in float32r (single-pass fp32 at full PE rate; inputs
    must be produced "rounded to f32r" — casts ride existing copies,
    activations, and gpsimd cast-DMAs).
  - keys arrive [s, k]; PE-transposed (fp32, exact) to keysT [k, s].
    Four [128,128] transposes share one PSUM bank; one [128,512] copy
    (alternating DVE/ACT) moves+casts each to SBUF.
  - pk.T psum tiles [a=128, s=512] = WkT @ keysT; the two s-halves are
    computed back-to-back with the same stationary WkT tile to amortize
    the f32r self-weight-load.
  - tanh fused on ScalarE (bias = (pq+bq+bk)[a] per-partition), output
    written directly as f32r.
  - scores [1, 512] = Ws.T @ tanh with Ws as the 1-column stationary
    (trivial weight load) and tanh as the moving operand.
  - softmax in natural layout on one partition: exp via ScalarE with
    accum_out giving the denominator for free; no max-subtraction
    (|scores| <= ||Ws||_1 <= 32 so fp32 exp cannot overflow).
  - attn.T [s=128, 8] for the context matmul is produced by a 4KB
    DRAM-bounce scatter DMA (gpsimd, casting to f32r); the context
    matmuls of batch b are emitted in the middle of batch b+1's PE
    stream so the bounce latency is hidden.
  - context [1, v=512] = attnT @ values, values in natural [s, v]
    layout cast to f32r during their gpsimd DMA load.
"""

import sys

if "/opt/trn_rl_repo" not in sys.path:
    sys.path.insert(0, "/opt/trn_rl_repo")

import numpy as np
from contextlib import ExitStack

import concourse.bass as bass
import concourse.tile as tile
from concourse import bacc, mybir
from concourse.bass_utils import run_bass_kernel_spmd
from concourse.masks import make_identity

F32 = mybir.dt.float32
F32R = mybir.dt.float32r
AF = mybir.ActivationFunctionType

NCORES = 8
B, S, D = 64, 1024, 1024  # D = QD = KD = VD = AD
NB = B // NCORES          # local batches per core
P = 128
KC = D // P               # 8 contraction chunks
AT = D // P               # 8 a-tiles
SB = S // P               # 8 s-blocks of 128
SH = S // 512             # 2 s-halves of 512


def _build_nc(repeat=1, do_transpose=True):
    nc = bacc.Bacc("TRN2", target_bir_lowering=False, debug=False)

    q_d = nc.dram_tensor("query_l", [NB, D], F32, kind="ExternalInput").ap()
    k_d = nc.dram_tensor("keys_l", [NB, S, D], F32, kind="ExternalInput").ap()
    v_d = nc.dram_tensor("values_l", [NB, S, D], F32, kind="ExternalInput").ap()
    wq_d = nc.dram_tensor("Wq", [D, D], F32, kind="ExternalInput").ap()
    wk_d = nc.dram_tensor("Wk", [D, D], F32, kind="ExternalInput").ap()
    bq_d = nc.dram_tensor("bq", [D], F32, kind="ExternalInput").ap()
    bk_d = nc.dram_tensor("bk", [D], F32, kind="ExternalInput").ap()
    ws_d = nc.dram_tensor("Ws", [D], F32, kind="ExternalInput").ap()
    ctx_d = nc.dram_tensor("context_l", [NB, D], F32, kind="ExternalOutput").ap()
    attn_d = nc.dram_tensor("attn_l", [NB, S], F32, kind="ExternalOutput").ap()

    with tile.TileContext(nc) as tc, ExitStack() as ctx:
        const = ctx.enter_context(tc.tile_pool(name="const", bufs=1))
        tpsum = ctx.enter_context(
            tc.tile_pool(name="tpsum", bufs=2, space=bass.MemorySpace.PSUM)
        )
        mpsum = ctx.enter_context(
            tc.tile_pool(name="mpsum", bufs=4, space=bass.MemorySpace.PSUM)
        )
        spsum = ctx.enter_context(
            tc.tile_pool(name="spsum", bufs=2, space=bass.MemorySpace.PSUM)
        )

        ident = const.tile([P, P], F32, tag="ident")
        make_identity(nc, ident)

        # Ws -> [p, at] fp32 -> f32r
        ws_f = const.tile([P, AT], F32, tag="ws_f")
        nc.sync.dma_start(ws_f, ws_d.rearrange("(a p) -> p a", p=P))
        ws_r = const.tile([P, AT], F32R, tag="ws_r")
        nc.vector.tensor_copy(ws_r, ws_f)

        # bq + bk -> [p, at] fp32
        bq_f = const.tile([P, AT], F32, tag="bq_f")
        bk_f = const.tile([P, AT], F32, tag="bk_f")
        nc.sync.dma_start(bq_f, bq_d.rearrange("(a p) -> p a", p=P))
        nc.sync.dma_start(bk_f, bk_d.rearrange("(a p) -> p a", p=P))
        bqk_f = const.tile([P, AT], F32, tag="bqk_f")
        nc.vector.tensor_add(bqk_f, bq_f, bk_f)

        # WkT (resident): WkT_all[:, kc, a] = Wk[a, kc*128 + p]
        WkT_all = const.tile([P, KC, D], F32R, tag="WkT_all")
        bias_all = const.tile([P, AT, NB], F32, tag="bias_all")  # pq+bq+bk [p, at, b]

        with ExitStack() as setup_ctx:
            wstage = setup_ctx.enter_context(tc.tile_pool(name="wstage", bufs=2))
            wqpool = setup_ctx.enter_context(tc.tile_pool(name="wqpool", bufs=1))

            for at in range(AT):
                wk_nat = wstage.tile([P, D], F32, tag="wnat")
                nc.sync.dma_start(wk_nat, wk_d[at * P : (at + 1) * P, :])
                for kc2 in range(KC // 2):
                    pst = tpsum.tile([P, 2, P], F32, tag="tp")
                    for h in range(2):
                        kc = 2 * kc2 + h
                        nc.tensor.transpose(
                            pst[:, h, :], wk_nat[:, kc * P : (kc + 1) * P], ident
                        )
                    for h in range(2):
                        kc = 2 * kc2 + h
                        dst = WkT_all[:, kc, at * P : (at + 1) * P]
                        if kc2 % 2 == 0:
                            nc.vector.tensor_copy(dst, pst[:, h, :])
                        else:
                            nc.scalar.copy(dst, pst[:, h, :])

            WqT_all = wqpool.tile([P, KC, D], F32R, tag="WqT_all")
            for at in range(AT):
                wq_nat = wstage.tile([P, D], F32, tag="wnat")
                nc.sync.dma_start(wq_nat, wq_d[at * P : (at + 1) * P, :])
                for kc2 in range(KC // 2):
                    pst = tpsum.tile([P, 2, P], F32, tag="tp")
                    for h in range(2):
                        qc = 2 * kc2 + h
                        nc.tensor.transpose(
                            pst[:, h, :], wq_nat[:, qc * P : (qc + 1) * P], ident
                        )
                    for h in range(2):
                        qc = 2 * kc2 + h
                        dst = WqT_all[:, qc, at * P : (at + 1) * P]
                        if kc2 % 2 == 0:
                            nc.vector.tensor_copy(dst, pst[:, h, :])
                        else:
                            nc.scalar.copy(dst, pst[:, h, :])

            # queryT [p(q), qc, b] f32r via PE transposes of query [NB, D]
            q_nat = wqpool.tile([NB, D], F32, tag="q_nat")
            nc.sync.dma_start(q_nat, q_d)
            qT = wqpool.tile([P, KC, NB], F32R, tag="qT")
            for qc in range(KC):
                pst = tpsum.tile([P, NB], F32, tag="tp")
                nc.tensor.transpose(
                    pst, q_nat[0:NB, qc * P : (qc + 1) * P], ident[0:NB, 0:NB]
                )
                nc.vector.tensor_copy(qT[:, qc, :], pst)

            # pqT [a, b] per a-tile; bias_all = pqT + (bq + bk)
            for at in range(AT):
                pqp = tpsum.tile([P, NB], F32, tag="tp")
                for qc in range(KC):
                    nc.tensor.matmul(
                        pqp,
                        WqT_all[:, qc, at * P : (at + 1) * P],
                        qT[:, qc, :],
                        start=(qc == 0),
                        stop=(qc == KC - 1),
                    )
                nc.vector.tensor_scalar_add(
                    bias_all[:, at, :], pqp, bqk_f[:, at : at + 1]
                )

        # ---- main loop over local batches ----
        kpool = ctx.enter_context(tc.tile_pool(name="kpool", bufs=4))
        ktpool = ctx.enter_context(tc.tile_pool(name="ktpool", bufs=2))
        thpool = ctx.enter_context(tc.tile_pool(name="thpool", bufs=16))
        vpool = ctx.enter_context(tc.tile_pool(name="vpool", bufs=10))
        smpool = ctx.enter_context(tc.tile_pool(name="smpool", bufs=2))
        outpool = ctx.enter_context(tc.tile_pool(name="outpool", bufs=2))

        rep_ctx = ExitStack()
        if repeat > 1:
            rep_ctx.enter_context(tc.For_i(0, repeat, 1))

        pending_ctx = None  # (attnT_r, vals, b) of previous batch

        def emit_ctx(pend):
            attnT_r, vals_, b_ = pend
            for vc in range(2):
                cps = spsum.tile([1, 512], F32, tag="sc")
                for sb in range(SB):
                    nc.tensor.matmul(
                        cps,
                        attnT_r[:, sb : sb + 1],
                        vals_[sb][:, vc * 512 : (vc + 1) * 512],
                        start=(sb == 0),
                        stop=(sb == SB - 1),
                    )
                ctx_sb = outpool.tile([1, 512], F32, tag="ctx_sb")
                nc.vector.tensor_copy(ctx_sb, cps)
                nc.sync.dma_start(ctx_d[b_, vc * 512 : (vc + 1) * 512], ctx_sb)

        for b in range(NB):
            # values prefetch (cast to f32r on the way in via SWDGE)
            vals = []
            for sb in range(SB):
                vt = vpool.tile([P, D], F32R, tag="vals")
                nc.gpsimd.dma_start(vt, v_d[b, sb * P : (sb + 1) * P, :])
                vals.append(vt)

            # keys load + transpose into keysT [k, s] per s-half
            kts = []
            for sh in range(SH):
                kt = ktpool.tile([P, KC, 512], F32R, tag="kt")
                knats = []
                for sb4 in range(4):
                    s0 = sh * 512 + sb4 * P
                    knat = kpool.tile([P, D], F32, tag="knat")
                    nc.sync.dma_start(knat, k_d[b, s0 : s0 + P, :])
                    knats.append(knat)
                for kc in range(KC):
                    if not do_transpose:
                        if kc % 2 == 0:
                            nc.vector.tensor_copy(kt[:, kc, :], knats[0][:, 0:512])
                        else:
                            nc.scalar.copy(kt[:, kc, :], knats[0][:, 0:512])
                        continue
                    pst = tpsum.tile([P, 512], F32, tag="tp")
                    for sb4 in range(4):
                        nc.tensor.transpose(
                            pst[:, sb4 * P : (sb4 + 1) * P],
                            knats[sb4][:, kc * P : (kc + 1) * P],
                            ident,
                        )
                    if kc % 2 == 0:
                        nc.vector.tensor_copy(kt[:, kc, :], pst)
                    else:
                        nc.scalar.copy(kt[:, kc, :], pst)
                kts.append(kt)

            # pk.T + tanh; s-halves paired to amortize weight loads
            th = [[None] * AT for _ in range(SH)]
            for at in range(AT):
                if at == 2 and pending_ctx is not None:
                    emit_ctx(pending_ctx)
                    pending_ctx = None
                mps = [
                    mpsum.tile([P, 512], F32, tag="mp", name=f"mp_{b}_{at}_{sh}")
                    for sh in range(SH)
                ]
                for kc in range(KC):
                    w = WkT_all[:, kc, at * P : (at + 1) * P]
                    for sh in range(SH):
                        nc.tensor.matmul(
                            mps[sh],
                            w,
                            kts[sh][:, kc, :],
                            start=(kc == 0),
                            stop=(kc == KC - 1),
                        )
                for sh in range(SH):
                    t = thpool.tile([P, 512], F32R, tag="th")
                    nc.scalar.activation(
                        t, mps[sh], AF.Tanh, bias=bias_all[:, at, b : b + 1]
                    )
                    th[sh][at] = t

            # scores [1, 512] per s-half: Ws (1-col stationary) vs tanh moving
            scores_sb = smpool.tile([1, S], F32, tag="scores")
            for sh in range(SH):
                scp = spsum.tile([1, 512], F32, tag="sc")
                for at in range(AT):
                    nc.tensor.matmul(
                        scp,
                        ws_r[:, at : at + 1],
                        th[sh][at],
                        start=(at == 0),
                        stop=(at == AT - 1),
                    )
                nc.vector.tensor_copy(scores_sb[:, sh * 512 : (sh + 1) * 512], scp)

            # softmax on one partition; denominator via accum_out
            exp_sb = smpool.tile([1, S], F32, tag="exp")
            den = smpool.tile([1, 1], F32, tag="den")
            nc.scalar.activation(exp_sb, scores_sb, AF.Exp, accum_out=den)
            rden = smpool.tile([1, 1], F32, tag="rden")
            nc.vector.reciprocal(rden, den)
            attn_f = outpool.tile([1, S], F32, tag="attn_f")
            nc.vector.tensor_scalar_mul(attn_f, exp_sb, rden)
            nc.sync.dma_start(attn_d[b, :], attn_f)

            # attn.T via DRAM bounce (scatter + f32r cast on SWDGE)
            attnT_r = smpool.tile([P, SB], F32R, tag="attnT")
            nc.gpsimd.dma_start(
                attnT_r, attn_d[b, :].rearrange("(sb p) -> p sb", p=P)
            )

            pending_ctx = (attnT_r, vals, b)

        emit_ctx(pending_ctx)
        rep_ctx.close()

    nc.compile()
    if not nc.is_finalized():
        nc.finalize()
    return nc


_NC_CACHE = None


def _get_nc():
    global _NC_CACHE
    if _NC_CACHE is None:
        _NC_CACHE = _build_nc()
    return _NC_CACHE


def kernel(query, keys, values, Wq, bq, Wk, bk, Ws, **kw):
    query = np.ascontiguousarray(np.asarray(query, dtype=np.float32))
    keys = np.ascontiguousarray(np.asarray(keys, dtype=np.float32))
    values = np.ascontiguousarray(np.asarray(values, dtype=np.float32))
    Wq = np.ascontiguousarray(np.asarray(Wq, dtype=np.float32))
    Wk = np.ascontiguousarray(np.asarray(Wk, dtype=np.float32))
    bq = np.ascontiguousarray(np.asarray(bq, dtype=np.float32))
    bk = np.ascontiguousarray(np.asarray(bk, dtype=np.float32))
    Ws = np.ascontiguousarray(np.asarray(Ws, dtype=np.float32))

    nc = _get_nc()
    in_maps = []
    for c in range(NCORES):
        lo, hi = c * NB, (c + 1) * NB
        in_maps.append(
            {
                "query_l": query[lo:hi],
                "keys_l": keys[lo:hi],
                "values_l": values[lo:hi],
                "Wq": Wq,
                "Wk": Wk,
                "bq": bq,
                "bk": bk,
                "Ws": Ws,
            }
        )
    res = run_bass_kernel_spmd(nc, in_maps, core_ids=list(range(NCORES)))
    context = np.concatenate([r["context_l"] for r in res.results], axis=0)
    attn = np.concatenate([r["attn_l"] for r in res.results], axis=0)
    return context, attn


# revision 13
# speedup vs baseline: 190.7482x; 175.0742x over previous
"""Bahdanau additive attention on 8 Trainium2 NeuronCores.

Problem (per reference):
    pq     = query @ Wq.T + bq                         [B, A]
    pk     = einsum("bsk,ak->bsa", keys, Wk) + bk      [B, S, A]
    scores = einsum("bsa,a->bs", tanh(pq[:,None,:] + pk), Ws)
    attn   = softmax(scores, axis=1)                   [B, S]
    context= einsum("bs,bsv->bv", attn, values)        [B, V]
    returns (context, attn)

B=64, S=1024, QD=KD=VD=AD=1024, fp32.

Sharding: data-parallel over batch across 8 cores (8 batches/core),
weights replicated. No collectives.

Per-core design:
  - All matmuls in float32r (single-pass fp32 at full PE rate; inputs
    must be produced "rounded to f32r" — casts ride existing copies,
    activations, and gpsimd cast-DMAs).
  - keys arrive [s, k]; PE-transposed (fp32, exact) to keysT [k, s].
    Four [128,128] transposes share one PSUM bank; one [128,512] copy
    (alternating DVE/ACT) moves+casts each to SBUF.
  - pk.T psum tiles [a=128, s=512] = WkT @ keysT; the two s-halves are
    computed back-to-back with the same stationary WkT tile to amortize
    the f32r self-weight-load.
  - tanh fused on ScalarE (bias = (pq+bq+bk)[a] per-partition), output
    written directly as f32r.
  - scores [1, 512] = Ws.T @ tanh with Ws as the 1-column stationary
    (trivial weight load) and tanh as the moving operand.
  - softmax in natural layout on one partition: exp via ScalarE with
    accum_out giving the denominator for free; no max-subtraction
    (|scores| <= ||Ws||_1 <= 32 so fp32 exp cannot overflow).
  - attn.T [s=128, 8] for the context matmul is produced by a 4KB
    DRAM-bounce scatter DMA (gpsimd, casting to f32r); the context
    matmuls of batch b are emitted in the middle of batch b+1's PE
    stream so the bounce latency is hidden.
  - context [1, v=512] = attnT @ values, values in natural [s, v]
    layout cast to f32r during their gpsimd DMA load.
"""

import sys

if "/opt/trn_rl_repo" not in sys.path:
    sys.path.insert(0, "/opt/trn_rl_repo")

import numpy as np
from contextlib import ExitStack

import concourse.bass as bass
import concourse.tile as tile
from concourse import bacc, mybir
from concourse.bass_utils import run_bass_kernel_spmd
from concourse.masks import make_identity

F32 = mybir.dt.float32
F32R = mybir.dt.float32r
AF = mybir.ActivationFunctionType

NCORES = 8
B, S, D = 64, 1024, 1024  # D = QD = KD = VD = AD
NB = B // NCORES          # local batches per core
P = 128
KC = D // P               # 8 contraction chunks
AT = D // P               # 8 a-tiles
SB = S // P               # 8 s-blocks of 128
SH = S // 512             # 2 s-halves of 512


def _build_nc(repeat=1, do_transpose=True):
    nc = bacc.Bacc("TRN2", target_bir_lowering=False, debug=False)

    q_d = nc.dram_tensor("query_l", [NB, D], F32, kind="ExternalInput").ap()
    k_d = nc.dram_tensor("keys_l", [NB, S, D], F32, kind="ExternalInput").ap()
    v_d = nc.dram_tensor("values_l", [NB, S, D], F32, kind="ExternalInput").ap()
    wq_d = nc.dram_tensor("Wq", [D, D], F32, kind="ExternalInput").ap()
    wk_d = nc.dram_tensor("Wk", [D, D], F32, kind="ExternalInput").ap()
    bq_d = nc.dram_tensor("bq", [D], F32, kind="ExternalInput").ap()
    bk_d = nc.dram_tensor("bk", [D], F32, kind="ExternalInput").ap()
    ws_d = nc.dram_tensor("Ws", [D], F32, kind="ExternalInput").ap()
    ctx_d = nc.dram_tensor("context_l", [NB, D], F32, kind="ExternalOutput").ap()
    attn_d = nc.dram_tensor("attn_l", [NB, S], F32, kind="ExternalOutput").ap()

    with tile.TileContext(nc) as tc, ExitStack() as ctx:
        const = ctx.enter_context(tc.tile_pool(name="const", bufs=1))
        tpsum = ctx.enter_context(
            tc.tile_pool(name="tpsum", bufs=2, space=bass.MemorySpace.PSUM)
        )
        mpsum = ctx.enter_context(
            tc.tile_pool(name="mpsum", bufs=4, space=bass.MemorySpace.PSUM)
        )
        spsum = ctx.enter_context(
            tc.tile_pool(name="spsum", bufs=2, space=bass.MemorySpace.PSUM)
        )

        ident = const.tile([P, P], F32, tag="ident")
        make_identity(nc, ident)

        # Ws -> [p, at] fp32 -> f32r
        ws_f = const.tile([P, AT], F32, tag="ws_f")
        nc.sync.dma_start(ws_f, ws_d.rearrange("(a p) -> p a", p=P))
        ws_r = const.tile([P, AT], F32R, tag="ws_r")
        nc.vector.tensor_copy(ws_r, ws_f)

        # bq + bk -> [p, at] fp32
        bq_f = const.tile([P, AT], F32, tag="bq_f")
        bk_f = const.tile([P, AT], F32, tag="bk_f")
        nc.sync.dma_start(bq_f, bq_d.rearrange("(a p) -> p a", p=P))
        nc.sync.dma_start(bk_f, bk_d.rearrange("(a p) -> p a", p=P))
        bqk_f = const.tile([P, AT], F32, tag="bqk_f")
        nc.vector.tensor_add(bqk_f, bq_f, bk_f)

        # WkT (resident): WkT_all[:, kc, a] = Wk[a, kc*128 + p]
        WkT_all = const.tile([P, KC, D], F32R, tag="WkT_all")
        bias_all = const.tile([P, AT, NB], F32, tag="bias_all")  # pq+bq+bk [p, at, b]

        with ExitStack() as setup_ctx:
            wstage = setup_ctx.enter_context(tc.tile_pool(name="wstage", bufs=2))
            wqpool = setup_ctx.enter_context(tc.tile_pool(name="wqpool", bufs=1))

            for at in range(AT):
                wk_nat = wstage.tile([P, D], F32, tag="wnat")
                nc.sync.dma_start(wk_nat, wk_d[at * P : (at + 1) * P, :])
                for kc2 in range(KC // 2):
                    pst = tpsum.tile([P, 2, P], F32, tag="tp")
                    for h in range(2):
                        kc = 2 * kc2 + h
                        nc.tensor.transpose(
                            pst[:, h, :], wk_nat[:, kc * P : (kc + 1) * P], ident
                        )
                    for h in range(2):
                        kc = 2 * kc2 + h
                        dst = WkT_all[:, kc, at * P : (at + 1) * P]
                        if kc2 % 2 == 0:
                            nc.vector.tensor_copy(dst, pst[:, h, :])
                        else:
                            nc.scalar.copy(dst, pst[:, h, :])

            WqT_all = wqpool.tile([P, KC, D], F32R, tag="WqT_all")
            for at in range(AT):
                wq_nat = wstage.tile([P, D], F32, tag="wnat")
                nc.sync.dma_start(wq_nat, wq_d[at * P : (at + 1) * P, :])
                for kc2 in range(KC // 2):
                    pst = tpsum.tile([P, 2, P], F32, tag="tp")
                    for h in range(2):
                        qc = 2 * kc2 + h
                        nc.tensor.transpose(
                            pst[:, h, :], wq_nat[:, qc * P : (qc + 1) * P], ident
                        )
                    for h in range(2):
                        qc = 2 * kc2 + h
                        dst = WqT_all[:, qc, at * P : (at + 1) * P]
                        if kc2 % 2 == 0:
                            nc.vector.tensor_copy(dst, pst[:, h, :])
                        else:
                            nc.scalar.copy(dst, pst[:, h, :])

            # queryT [p(q), qc, b] f32r via PE transposes of query [NB, D]
            q_nat = wqpool.tile([NB, D], F32, tag="q_nat")
            nc.sync.dma_start(q_nat, q_d)
            qT = wqpool.tile([P, KC, NB], F32R, tag="qT")
            for qc in range(KC):
                pst = tpsum.tile([P, NB], F32, tag="tp")
                nc.tensor.transpose(
                    pst, q_nat[0:NB, qc * P : (qc + 1) * P], ident[0:NB, 0:NB]
                )
                nc.vector.tensor_copy(qT[:, qc, :], pst)

            # pqT [a, b] per a-tile; bias_all = pqT + (bq + bk)
            for at in range(AT):
                pqp = tpsum.tile([P, NB], F32, tag="tp")
                for qc in range(KC):
                    nc.tensor.matmul(
                        pqp,
                        WqT_all[:, qc, at * P : (at + 1) * P],
                        qT[:, qc, :],
                        start=(qc == 0),
                        stop=(qc == KC - 1),
                    )
                nc.vector.tensor_scalar_add(
                    bias_all[:, at, :], pqp, bqk_f[:, at : at + 1]
                )

        # ---- main loop over local batches ----
        kpool = ctx.enter_context(tc.tile_pool(name="kpool", bufs=4))
        ktpool = ctx.enter_context(tc.tile_pool(name="ktpool", bufs=4))
        thpool = ctx.enter_context(tc.tile_pool(name="thpool", bufs=16))
        vpool = ctx.enter_context(tc.tile_pool(name="vpool", bufs=8))
        smpool = ctx.enter_context(tc.tile_pool(name="smpool", bufs=2))
        outpool = ctx.enter_context(tc.tile_pool(name="outpool", bufs=2))

        rep_ctx = ExitStack()
        if repeat > 1:
            rep_ctx.enter_context(tc.For_i(0, repeat, 1))

        pending_ctx = None  # (attnT_r, vals, b) of previous batch
        kts_next = None     # keysT tiles of the next batch, filled a quarter
                            # at a time interleaved into this batch's PE stream

        def emit_ctx(pend):
            attnT_r, vals_, b_ = pend
            for vc in range(2):
                cps = spsum.tile([1, 512], F32, tag="sc")
                for sb in range(SB):
                    nc.tensor.matmul(
                        cps,
                        attnT_r[:, sb : sb + 1],
                        vals_[sb][:, vc * 512 : (vc + 1) * 512],
                        start=(sb == 0),
                        stop=(sb == SB - 1),
                    )
                ctx_sb = outpool.tile([1, 512], F32, tag="ctx_sb")
                nc.vector.tensor_copy(ctx_sb, cps)
                nc.sync.dma_start(ctx_d[b_, vc * 512 : (vc + 1) * 512], ctx_sb)

        def alloc_kts(b):
            return [
                ktpool.tile([P, KC, 512], F32R, tag="kt", name=f"kt_{b}_{sh}")
                for sh in range(SH)
            ]

        def emit_keys_quarter(kts, b, q):
            """Load + transpose s-range [q*256, (q+1)*256) of batch b into
            kts[q//2][:, :, (q%2)*256 : (q%2+1)*256]."""
            sh, half = q // 2, q % 2
            knats = []
            for j in range(2):
                s0 = q * 256 + j * P
                knat = kpool.tile([P, D], F32, tag="knat", name=f"knat_{b}_{q}_{j}")
                nc.sync.dma_start(knat, k_d[b % NB, s0 : s0 + P, :])
                knats.append(knat)
            for kc in range(KC):
                if not do_transpose:
                    dst = kts[sh][:, kc, half * 256 : (half + 1) * 256]
                    if kc % 2 == 0:
                        nc.vector.tensor_copy(dst, knats[0][:, 0:256])
                    else:
                        nc.scalar.copy(dst, knats[0][:, 0:256])
                    continue
                pst = tpsum.tile([P, 256], F32, tag="tp")
                for j in range(2):
                    nc.tensor.transpose(
                        pst[:, j * P : (j + 1) * P],
                        knats[j][:, kc * P : (kc + 1) * P],
                        ident,
                    )
                dst = kts[sh][:, kc, half * 256 : (half + 1) * 256]
                if kc % 2 == 0:
                    nc.vector.tensor_copy(dst, pst)
                else:
                    nc.scalar.copy(dst, pst)

        # preamble: batch 0's keysT built up front
        kts_cur = alloc_kts(0)
        for q in range(4):
            emit_keys_quarter(kts_cur, 0, q)

        for b in range(NB):
            # values prefetch (cast to f32r on the way in via SWDGE)
            vals = []
            for sb in range(SB):
                vt = vpool.tile([P, D], F32R, tag="vals", name=f"vals_{b}_{sb}")
                nc.gpsimd.dma_start(vt, v_d[b, sb * P : (sb + 1) * P, :])
                vals.append(vt)

            emit_next = (b < NB - 1) or (repeat > 1)
            if emit_next:
                kts_next = alloc_kts(b + 1)

            # pk.T + tanh; s-halves paired to amortize weight loads;
            # next batch's transposes interleaved between at-groups
            th = [[None] * AT for _ in range(SH)]
            for at in range(AT):
                if at == 2 and pending_ctx is not None:
                    emit_ctx(pending_ctx)
                    pending_ctx = None
                mps = [
                    mpsum.tile([P, 512], F32, tag="mp", name=f"mp_{b}_{at}_{sh}")
                    for sh in range(SH)
                ]
                for kc in range(KC):
                    w = WkT_all[:, kc, at * P : (at + 1) * P]
                    for sh in range(SH):
                        nc.tensor.matmul(
                            mps[sh],
                            w,
                            kts_cur[sh][:, kc, :],
                            start=(kc == 0),
                            stop=(kc == KC - 1),
                        )
                for sh in range(SH):
                    t = thpool.tile([P, 512], F32R, tag="th", name=f"th_{b}_{at}_{sh}")
                    nc.scalar.activation(
                        t, mps[sh], AF.Tanh, bias=bias_all[:, at, b : b + 1]
                    )
                    th[sh][at] = t
                if emit_next and at % 2 == 1:
                    emit_keys_quarter(kts_next, b + 1, at // 2)

            # scores [1, 512] per s-half: Ws (1-col stationary) vs tanh moving
            scex = smpool.tile([1, S], F32, tag="scex", name=f"scex_{b}")
            for sh in range(SH):
                scp = spsum.tile([1, 512], F32, tag="sc")
                for at in range(AT):
                    nc.tensor.matmul(
                        scp,
                        ws_r[:, at : at + 1],
                        th[sh][at],
                        start=(at == 0),
                        stop=(at == AT - 1),
                    )
                nc.vector.tensor_copy(scex[:, sh * 512 : (sh + 1) * 512], scp)

            # softmax on one partition, in place; denominator via accum_out
            den = smpool.tile([1, 1], F32, tag="den")
            nc.scalar.activation(scex, scex, AF.Exp, accum_out=den)
            rden = smpool.tile([1, 1], F32, tag="rden")
            nc.vector.reciprocal(rden, den)
            nc.vector.tensor_scalar_mul(scex, scex, rden)
            nc.sync.dma_start(attn_d[b, :], scex)

            # attn.T via DRAM bounce (scatter + f32r cast on SWDGE)
            attnT_r = smpool.tile([P, SB], F32R, tag="attnT", name=f"attnT_{b}")
            nc.gpsimd.dma_start(
                attnT_r, attn_d[b, :].rearrange("(sb p) -> p sb", p=P)
            )

            pending_ctx = (attnT_r, vals, b)
            kts_cur = kts_next

        emit_ctx(pending_ctx)
        rep_ctx.close()

    nc.compile()
    if not nc.is_finalized():
        nc.finalize()
    return nc


_NC_CACHE = None


def _get_nc():
    global _NC_CACHE
    if _NC_CACHE is None:
        _NC_CACHE = _build_nc()
    return _NC_CACHE


def kernel(query, keys, values, Wq, bq, Wk, bk, Ws, **kw):
    query = np.ascontiguousarray(np.asarray(query, dtype=np.float32))
    keys = np.ascontiguousarray(np.asarray(keys, dtype=np.float32))
    values = np.ascontiguousarray(np.asarray(values, dtype=np.float32))
    Wq = np.ascontiguousarray(np.asarray(Wq, dtype=np.float32))
    Wk = np.ascontiguousarray(np.asarray(Wk, dtype=np.float32))
    bq = np.ascontiguousarray(np.asarray(bq, dtype=np.float32))
    bk = np.ascontiguousarray(np.asarray(bk, dtype=np.float32))
    Ws = np.ascontiguousarray(np.asarray(Ws, dtype=np.float32))

    nc = _get_nc()
    in_maps = []
    for c in range(NCORES):
        lo, hi = c * NB, (c + 1) * NB
        in_maps.append(
            {
                "query_l": query[lo:hi],
                "keys_l": keys[lo:hi],
                "values_l": values[lo:hi],
                "Wq": Wq,
                "Wk": Wk,
                "bq": bq,
                "bk": bk,
                "Ws": Ws,
            }
        )
    res = run_bass_kernel_spmd(nc, in_maps, core_ids=list(range(NCORES)))
    context = np.concatenate([r["context_l"] for r in res.results], axis=0)
    attn = np.concatenate([r["attn_l"] for r in res.results], axis=0)
    return context, attn
